# revision 1
# baseline (speedup 1.0000x reference)
"""Trainium2 Bass kernel for nn_DSnetwork (GNN message passing), 8-core SPMD.

Strategy (data-parallel over graphs, per the sharding hint):
  - 1024 graphs per core; each core's subgraph rows are packed into 33
    "units" of 1024 rows (whole graphs per unit, zero-padded), giving a
    fixed-shape SPMD program; per-graph "slots" (64 per unit) relabel
    graphs so all addresses are uniform across cores.
  - h kept feature-major [128, R] in SBUF (bf16). Per layer:
      * DMA-transpose h -> row-major staging; PE computes per-unit
        segment MEANS via a weighted one-hot (values 1/cnt) matmul.
      * x2 = mean @ W_sum + bias via PE, per 128-slot window.
      * z = h @ W_fc + x2[slots] (+1, biases folded) accumulated in PSUM
        via two matmuls; ELU via H = min(max(z+1,1), exp(z)) where
        H = elu(z)+1 (the +1 is corrected in the next layer's biases).
  - Head computed feature-major per 128-slot window; host reassembles
    [8192, 10] from per-core slot outputs.
"""

import sys

sys.path.insert(0, "/opt/trn_rl_repo")

from contextlib import ExitStack

import numpy as np
import ml_dtypes

import concourse.mybir as mybir
import concourse.bacc as bacc
import concourse.tile as tile

BF = ml_dtypes.bfloat16
F8 = ml_dtypes.float8_e4m3
DT_BF = mybir.dt.bfloat16
DT_F8 = mybir.dt.float8e4
DT_F32 = mybir.dt.float32
OP = mybir.AluOpType
AF = mybir.ActivationFunctionType

# Problem constants (hardcoded per contest rules)
G, D, L, NT = 8192, 128, 3, 10
NC, GPC = 8, 1024
UNIT, U = 1024, 33          # rows per unit, units per core
RPAD = U * UNIT             # 33792 padded rows per core
SPU = 64                    # slots per unit (63 real + 1 trash)
NSLOT = U * SPU             # 2112
NGRP = (U + 1) // 2         # 17 gather groups (2 units each; last has 1)
NSLOTW = NGRP * 128         # 2176 (g_fm padded width)
ZCH = 1536                  # z-chunk columns (3 PSUM banks)
NZ = RPAD // ZCH            # 22
NBLK = RPAD // 128          # 264 row-blocks
NLOADCH = 11                # h_fm load chunks (3072 cols each)
TRU = 2                     # units per DMA-transpose instruction

_compiled = {}


def _host_prep(inputs):
    idx = np.asarray(inputs["subgraph_idx"]).astype(np.int64)
    h = np.asarray(inputs["h_subgraph"], dtype=np.float32)
    cnt = np.bincount(idx, minlength=G)
    assert cnt.min() >= 1, "zero-count graphs unsupported by bias folding"
    off = np.zeros(G + 1, np.int64)
    off[1:] = np.cumsum(cnt)

    Wfc = np.asarray(inputs["W_fc"], np.float32)
    bfc = np.asarray(inputs["b_fc"], np.float32)
    Wsum = np.asarray(inputs["W_sum"], np.float32)
    bsum = np.asarray(inputs["b_sum"], np.float32)
    Wf1 = np.asarray(inputs["W_f1"], np.float32)
    bf1 = np.asarray(inputs["b_f1"], np.float32)
    Wf2 = np.asarray(inputs["W_f2"], np.float32)
    bf2 = np.asarray(inputs["b_f2"], np.float32)

    Wfc_t = np.concatenate([Wfc[l] for l in range(L)], axis=1).astype(BF)
    Wsum_t = np.concatenate([Wsum[l] for l in range(L)], axis=1).astype(np.float32)
    bias_cols = []
    for l in range(L):
        b = bsum[l] + bfc[l]
        if l >= 1:
            b = b - Wsum[l].sum(axis=0) - Wfc[l].sum(axis=0)
        bias_cols.append(b)
    biasA_t = np.stack(bias_cols, axis=1).astype(np.float32)      # [128, L]
    biasB_t = biasA_t + 1.0                                       # [128, L]
    Wf1_t = Wf1.astype(np.float32)                                # [128, 256]
    bh1_t = np.stack([(bf1 - Wf1.sum(axis=0))[hh * 128:(hh + 1) * 128]
                      for hh in range(2)], axis=1).astype(np.float32)  # [128, 2]
    Wf2_t = np.concatenate([Wf2[0:128], Wf2[128:256]], axis=1).astype(np.float32)  # [128, 20]
    bh2_t = np.tile(bf2[None, :], (128, 1)).astype(np.float32)    # [128, 10]

    shared = {"Wfc": Wfc_t, "Wsum": Wsum_t, "biasA": biasA_t, "biasB": biasB_t,
              "Wf1": Wf1_t, "bh1": bh1_t, "Wf2": Wf2_t, "bh2": bh2_t}

    in_maps, slotmaps = [], []
    for c in range(NC):
        g0 = c * GPC
        units, cur, cur_rows = [], [], 0
        for g in range(g0, g0 + GPC):
            r = int(cnt[g])
            if cur_rows + r > UNIT or len(cur) + 1 > SPU - 1:
                units.append(cur)
                cur, cur_rows = [], 0
            cur.append(g)
            cur_rows += r
        units.append(cur)
        assert len(units) <= U, f"core {c}: needs {len(units)} units > {U}"
        units += [[] for _ in range(U - len(units))]

        hfm = np.zeros((RPAD, D), np.float32)
        slotrel = np.full(RPAD, SPU - 1, np.int32)      # pad rows -> trash slot
        ow = np.zeros(RPAD, np.float32)                  # one-hot weight (1/cnt; 0 for pad)
        s2g = np.full(NSLOT, -1, np.int64)
        for u, gs in enumerate(units):
            rr = u * UNIT
            for k, g in enumerate(gs):
                n = int(cnt[g])
                hfm[rr:rr + n] = h[off[g]:off[g] + n]
                slotrel[rr:rr + n] = k
                ow[rr:rr + n] = 1.0 / n
                s2g[u * SPU + k] = g
                rr += n

        # orm: [128, NBLK*64]; block b, partition p (= row 128b+p), col q:
        #   1/cnt if slotrel[row] == q (0 for pad rows since ow=0)
        sr = slotrel.reshape(NBLK, 128)                  # [b, p]
        oww = ow.reshape(NBLK, 128)
        orm = (sr[:, :, None] == np.arange(SPU)[None, None, :]) * oww[:, :, None]
        orm = np.ascontiguousarray(orm.transpose(1, 0, 2).reshape(128, NBLK * SPU)).astype(BF)

        # oT: [128, RPAD]; oT[p, j] = (64*(unit(j)%2) + slotrel[j] == p)
        uofj = np.arange(RPAD) // UNIT
        winslot = 64 * (uofj % 2) + slotrel
        oT = np.ascontiguousarray(
            (winslot[None, :] == np.arange(128)[:, None])).astype(F8)

        m = {"hfm": np.ascontiguousarray(hfm.T).astype(BF),
             "orm": orm, "oT": oT}
        m.update(shared)
        in_maps.append(m)
        slotmaps.append(s2g)
    return in_maps, slotmaps


def _build_nc(reps=1, loop_n=None):
    nc = bacc.Bacc("TRN2", target_bir_lowering=False, debug=False, num_devices=NC)
    A = {}
    for name, shape, dt in [
        ("hfm", [128, RPAD], DT_BF), ("orm", [128, NBLK * SPU], DT_BF),
        ("oT", [128, RPAD], DT_F8),
        ("Wfc", [128, L * 128], DT_BF), ("Wsum", [128, L * 128], DT_F32),
        ("biasA", [128, L], DT_F32), ("biasB", [128, L], DT_F32),
        ("Wf1", [128, 256], DT_F32),
        ("bh1", [128, 2], DT_F32), ("Wf2", [128, 2 * NT], DT_F32),
        ("bh2", [128, NT], DT_F32),
    ]:
        A[name] = nc.dram_tensor(name, shape, dt, kind="ExternalInput").ap()
    out_d = nc.dram_tensor("out", [NSLOTW, NT], DT_F32, kind="ExternalOutput").ap()

    with tile.TileContext(nc) as tc, ExitStack() as ctx:
        pers = ctx.enter_context(tc.tile_pool(name="pers", bufs=1))
        hrm_pool = ctx.enter_context(tc.tile_pool(name="hrm", bufs=2))
        e_pool = ctx.enter_context(tc.tile_pool(name="ep", bufs=8))
        rr_pool = ctx.enter_context(tc.tile_pool(name="rrp", bufs=2))
        x2_pool = ctx.enter_context(tc.tile_pool(name="x2p", bufs=12))
        hd_pool = ctx.enter_context(tc.tile_pool(name="hd", bufs=2))
        zp = ctx.enter_context(tc.tile_pool(name="zp", bufs=2, space="PSUM"))
        mp = ctx.enter_context(tc.tile_pool(name="mp", bufs=2, space="PSUM"))

        hfm = pers.tile([128, RPAD], DT_BF, tag="hfm")
        oT = pers.tile([128, RPAD], DT_F8, tag="oT")
        orm = pers.tile([128, NBLK * SPU], DT_BF, tag="orm")
        gfm = pers.tile([128, NSLOTW], DT_F32, tag="gfm")
        Wfc_s = pers.tile([128, L * 128], DT_BF, tag="Wfc")
        Wsum_s = pers.tile([128, L * 128], DT_F32, tag="Wsum")
        biasA_s = pers.tile([128, L], DT_F32, tag="biasA")
        biasB_s = pers.tile([128, L], DT_F32, tag="biasB")
        Wf1_s = pers.tile([128, 256], DT_F32, tag="Wf1")
        bh1_s = pers.tile([128, 2], DT_F32, tag="bh1")
        Wf2_s = pers.tile([128, 2 * NT], DT_F32, tag="Wf2")
        bh2_s = pers.tile([128, NT], DT_F32, tag="bh2")

        for nm, t in [("Wfc", Wfc_s), ("Wsum", Wsum_s), ("biasA", biasA_s),
                      ("biasB", biasB_s), ("Wf1", Wf1_s), ("bh1", bh1_s),
                      ("Wf2", Wf2_s), ("bh2", bh2_s)]:
            nc.sync.dma_start(t[:], A[nm])
        nc.vector.memset(gfm[:, NSLOT:], 0.0)

        def load_pair(t):
            u0 = 2 * t
            nun = min(2, U - u0)
            c0, c1 = u0 * UNIT, (u0 + nun) * UNIT
            nc.sync.dma_start(hfm[:, c0:c1], A["hfm"][:, c0:c1])
            r0, r1 = u0 * 8 * SPU, (u0 + nun) * 8 * SPU
            nc.scalar.dma_start(orm[:, r0:r1], A["orm"][:, r0:r1])
            nc.scalar.dma_start(oT[:, c0:c1], A["oT"][:, c0:c1])

        hrm_hold = [None]
        pg_hold = [None]

        def seg_pair(t):
            """Segment-mean matmuls + evac for pair t; the DMA-transpose is
            batched 2 pairs (4 units) at a time on even t."""
            u0 = 2 * t
            nun = min(2, U - u0)
            if t % 2 == 0:
                n4 = min(4, U - u0)
                hrm = hrm_pool.tile([128, 2 * TRU * UNIT], DT_BF, tag="hrm")
                nc.sync.dma_start_transpose(
                    hrm[:, :n4 * UNIT].rearrange("p (b q) -> p b q", q=128),
                    hfm[:, u0 * UNIT:(u0 + n4) * UNIT])
                hrm_hold[0] = hrm
                boff = 0
            else:
                hrm = hrm_hold[0]
                boff = 2 * 8
            if t % 2 == 0:
                pgq = mp.tile([128, 2 * TRU * SPU], DT_F32, tag="m")
                pg_hold[0] = pgq
            pg = pg_hold[0]
            poff = 0 if t % 2 == 0 else TRU * SPU
            for uu in range(nun):
                u = u0 + uu
                for b in range(8):
                    nc.tensor.matmul(
                        pg[:, poff + uu * SPU:poff + (uu + 1) * SPU],
                        hrm[:, (boff + uu * 8 + b) * 128:(boff + uu * 8 + b + 1) * 128],
                        orm[:, (u * 8 + b) * SPU:(u * 8 + b + 1) * SPU],
                        start=(b == 0), stop=(b == 7))
            if t % 2 == 1 or t == NGRP - 1:
                us = (t // 2) * 2 * TRU
                nu = min(2 * TRU, U - us)
                nc.vector.tensor_copy(gfm[:, us * SPU:(us + nu) * SPU],
                                      pg[:, :nu * SPU])

        def x2_pair(l, t, x2s):
            px = mp.tile([128, 128], DT_F32, tag="m")
            nc.tensor.matmul(px[:], gfm[:, t * 128:(t + 1) * 128],
                             Wsum_s[:, l * 128:(l + 1) * 128],
                             start=True, stop=True)
            x2w = x2_pool.tile([128, 128], DT_BF, tag="x2w")
            nc.scalar.copy(x2w[:], px[:])
            x2s[t] = x2w

        def main_chunk(l, k, x2s):
            z = zp.tile([128, ZCH], DT_F32, tag="z")
            for s in range(3):
                t = k * 3 + s
                nc.tensor.matmul(z[:, s * 512:(s + 1) * 512],
                                 Wfc_s[:, l * 128:(l + 1) * 128],
                                 hfm[:, t * 512:(t + 1) * 512],
                                 start=True, stop=False)
            for s in range(3):
                t = k * 3 + s
                w = min(t // 4, NGRP - 1)
                nc.tensor.matmul(z[:, s * 512:(s + 1) * 512], x2s[w][:],
                                 oT[:, t * 512:(t + 1) * 512],
                                 start=False, stop=True)
            e = e_pool.tile([128, ZCH], DT_BF, tag="e")
            nc.scalar.activation(e[:], z[:], AF.Exp,
                                 bias=biasA_s[:, l:l + 1], scale=1.0)
            # H = min(max(z_true+1, 1), exp(z_true)) = elu(z_true)+1
            cols = hfm[:, k * ZCH:(k + 1) * ZCH]
            if k % 4 < 1:
                # path A: relu on ACT, fused combine on DVE
                rr = rr_pool.tile([128, ZCH], DT_BF, tag="rr")
                nc.scalar.activation(rr[:], z[:], AF.Relu,
                                     bias=biasA_s[:, l:l + 1], scale=1.0)
                nc.vector.scalar_tensor_tensor(cols, rr[:], 1.0, e[:],
                                               OP.add, OP.min)
            else:
                # path B: all-DVE
                nc.vector.tensor_scalar(cols, z[:], biasB_s[:, l:l + 1], 1.0,
                                        OP.add, OP.max)
                nc.vector.tensor_tensor(cols, cols, e[:], op=OP.min)

        def head_group(w):
            t1f = hd_pool.tile([128, 256], DT_F32, tag="t1f")
            for hh in range(2):
                p1 = mp.tile([128, 128], DT_F32, tag="m")
                nc.tensor.matmul(p1[:], Wf1_s[:, hh * 128:(hh + 1) * 128],
                                 gfm[:, w * 128:(w + 1) * 128],
                                 start=True, stop=True)
                nc.scalar.activation(t1f[:, hh * 128:(hh + 1) * 128], p1[:],
                                     AF.Relu, bias=bh1_s[:, hh:hh + 1], scale=1.0)
            po = mp.tile([128, NT], DT_F32, tag="m")
            nc.tensor.matmul(po[:], t1f[:, 0:128], Wf2_s[:, 0:NT],
                             start=True, stop=False)
            nc.tensor.matmul(po[:], t1f[:, 128:256], Wf2_s[:, NT:2 * NT],
                             start=False, stop=True)
            ob = hd_pool.tile([128, NT], DT_F32, tag="ob")
            nc.vector.tensor_add(ob[:], po[:], bh2_s[:])
            nc.sync.dma_start(out_d[w * 128:(w + 1) * 128, :], ob[:])

        def _emit_pipeline():
            # Prologue: loads + layer-0 seg/x2 pair-interleaved, with layer-0
            # main chunks emitted as soon as their inputs are in flight.
            x2s = [None] * NGRP
            k0 = 0
            lt = 0
            for t in range(NGRP):
                while lt <= min(t + 1, NGRP - 1):
                    load_pair(lt)
                    lt += 1
                seg_pair(t)
                if t % 2 == 1 or t == NGRP - 1:
                    for tt in ([t - 1, t] if t % 2 == 1 else [t]):
                        x2_pair(0, tt, x2s)
                x2d = t if (t % 2 == 1 or t == NGRP - 1) else t - 1
                while (k0 < NZ and (k0 + 1) * ZCH <= (t + 1) * TRU * UNIT
                       and min((3 * k0 + 2) // 4, NGRP - 1) <= x2d):
                    main_chunk(0, k0, x2s)
                    k0 += 1

            # Layers: finish this layer's main sweep while interleaving the
            # next layer's seg/x2 (or final seg + head) as columns finalize.
            for l in range(L):
                x2s_next = [None] * NGRP
                nxt = 0
                for k in range(k0 if l == 0 else 0, NZ):
                    main_chunk(l, k, x2s)
                    while nxt < NGRP and (nxt + (2 if nxt % 2 == 0 else 1)) \
                            * TRU * UNIT <= (k + 1) * ZCH:
                        seg_pair(nxt)
                        if nxt % 2 == 1 or nxt == NGRP - 1:
                            for tt in ([nxt - 1, nxt] if nxt % 2 == 1 else [nxt]):
                                if l < L - 1:
                                    x2_pair(l + 1, tt, x2s_next)
                                else:
                                    head_group(tt)
                        nxt += 1
                while nxt < NGRP:
                    seg_pair(nxt)
                    if nxt % 2 == 1 or nxt == NGRP - 1:
                        for tt in ([nxt - 1, nxt] if nxt % 2 == 1 else [nxt]):
                            if l < L - 1:
                                x2_pair(l + 1, tt, x2s_next)
                            else:
                                head_group(tt)
                    nxt += 1
                x2s = x2s_next

        if loop_n is not None:
            with tc.For_i(0, loop_n, 1):
                _emit_pipeline()
        else:
            for _rep in range(reps):
                _emit_pipeline()

    nc.compile()
    return nc


def get_nc(reps=1, loop_n=None):
    key = f"nc{reps}_{loop_n}"
    if key not in _compiled:
        _compiled[key] = _build_nc(reps, loop_n)
    return _compiled[key]


def kernel(**inputs) -> np.ndarray:
    in_maps, slotmaps = _host_prep(inputs)
    nc = get_nc()
    from concourse.bass_utils import run_bass_kernel_spmd
    res = run_bass_kernel_spmd(nc, in_maps, core_ids=list(range(NC)))
    full = np.zeros((G, NT), np.float32)
    for c in range(NC):
        oc = np.asarray(res.results[c]["out"], np.float32)[:NSLOT]
        s2g = slotmaps[c]
        valid = s2g >= 0
        full[s2g[valid]] = oc[valid]
    return full

